# revision 23
# baseline (speedup 1.0000x reference)
"""Trainium2 Bass kernel for LLN+diag attention.

out = 0.5 * (lln_linear_attention(q,k,v) + block_diag_attention(q,k,v))

Shapes: q,k,v [4,16,4096,64] fp32.  8 NeuronCores, one (B*H)/8 = 8-head
shard per core; both paths are independent per head so there is no
cross-device communication.

Host prep (sharding/layout only): the two global scalars sigma_q/sigma_k
(std over the whole tensor, inherently cross-device) are folded into the
shipped operands, which are also pre-transposed where the PE needs
d-major layout:
  qt = (alpha*q)^T      bf16 [.., 64, 4096]   (exp -> lin Q; also scores)
  kt = (k/(8*alpha))^T  bf16 [.., 64, 4096]   (scores: qt*kt = q*k/8)
  kb = beta*k           fp32 [.., 4096, 64]   (exp -> lin K)
  vb = v                bf16 [.., 4096, 64]
Math identities used on device:
  - row-max / global-max subtraction before exp cancels exactly in both
    paths' ratios (numerator and denominator scale together), and all
    exponents are <= ~12.5 so fp32 never overflows; EPS=1e-8 is ~1e-9
    relative to S and is dropped.
  - the "ones" column appended to V carries value 2.0, so each path's
    denominator is doubled -> the final add of the two halves is the
    required 0.5*(lin+diag).
"""

import math
import os
import sys

for _p in ("/opt/trn_rl_repo", "/opt/pypackages"):
    if os.path.isdir(_p) and _p not in sys.path:
        sys.path.insert(0, _p)

import numpy as np
import ml_dtypes

B, H, N, D = 4, 16, 4096, 64
N_CORES = 8
HPC = (B * H) // N_CORES          # heads per core = 8
NT = N // 128                     # 128-row n-tiles per head = 32
GROUPS = 8                        # groups per head
GNT = NT // GROUPS                # n-tiles per group = 4
A_CONST = 0.14855178144710912
B_CONST = -0.35487039130661086

_BF16 = ml_dtypes.bfloat16

_cache = {}


def _build():
    import concourse.bass as bass
    import concourse.bacc as bacc
    import concourse.mybir as mybir
    from concourse.tile import TileContext

    dt = mybir.dt
    F32, BF = dt.float32, dt.bfloat16
    Exp = mybir.ActivationFunctionType.Exp
    Copy = mybir.ActivationFunctionType.Copy
    MUL = mybir.AluOpType.mult
    ADD = mybir.AluOpType.add

    nc = bacc.Bacc()
    qt_d = nc.dram_tensor("qt", [HPC // 2, 128, N], BF, kind="ExternalInput")
    kt_d = nc.dram_tensor("kt", [HPC // 2, 128, N], BF, kind="ExternalInput")
    kb_d = nc.dram_tensor("kb", [HPC, 128, NT, D], BF, kind="ExternalInput")
    vb_d = nc.dram_tensor("vb", [HPC, 128, NT, D + 1], BF, kind="ExternalInput")
    out_d = nc.dram_tensor("out", [HPC, 128, NT, D], BF, kind="ExternalOutput")

    with TileContext(nc) as tc:
        from contextlib import ExitStack

        with ExitStack() as ctx:
            pair_p = ctx.enter_context(tc.tile_pool(name="pair", bufs=3))
            kb_p = ctx.enter_context(tc.tile_pool(name="kbp", bufs=3))
            head_p = ctx.enter_context(tc.tile_pool(name="head", bufs=3))
            out_p = ctx.enter_context(tc.tile_pool(name="outp", bufs=4))
            sm_p = ctx.enter_context(tc.tile_pool(name="small", bufs=4))
            at_p = ctx.enter_context(tc.tile_pool(name="attn", bufs=3))
            t_p = ctx.enter_context(tc.tile_pool(name="tmp", bufs=4))
            r_p = ctx.enter_context(tc.tile_pool(name="recip", bufs=8))
            kv_ps_p = ctx.enter_context(tc.tile_pool(name="kvps", bufs=1, space="PSUM"))
            sc_ps_p = ctx.enter_context(tc.tile_pool(name="scps", bufs=1, space="PSUM"))
            da_ps_p = ctx.enter_context(tc.tile_pool(name="daps", bufs=1, space="PSUM"))
            li_ps_p = ctx.enter_context(tc.tile_pool(name="lips", bufs=1, space="PSUM"))

            for p in range(HPC // 2):  # head pairs; heads 2p (parts 0:64), 2p+1 (64:128)
                qt2 = pair_p.tile([128, N], BF, tag="qt2")
                nc.sync.dma_start(qt2[:], qt_d[p])
                kt2 = pair_p.tile([128, N], BF, tag="kt2")
                nc.sync.dma_start(kt2[:], kt_d[p])
                qte2 = pair_p.tile([128, N], BF, tag="qte2")
                nc.scalar.activation(qte2[:], qt2[:], Exp)

                kes, vas, outs, kvas = [], [], [], []
                for hh in range(2):
                    h = 2 * p + hh
                    kb_t = kb_p.tile([128, NT, D], BF, tag="kb")
                    nc.sync.dma_start(kb_t[:], kb_d[h])
                    ke = head_p.tile([128, NT, D], BF, tag="ke")
                    nc.scalar.activation(ke[:], kb_t[:], Exp)
                    va = head_p.tile([128, NT, D + 1], BF, tag="va")
                    nc.sync.dma_start(va[:], vb_d[h])
                    kes.append(ke)
                    vas.append(va)
                    outs.append(out_p.tile([128, NT, D], BF, tag="oh", name="oh"))

                    # KV_aug[d, e|S] accumulated over all 32 n-tiles.
                    kv_ps = kv_ps_p.tile([128, D + 1], F32, tag=f"kv{hh}")
                    for a in range(NT):
                        nc.tensor.matmul(
                            kv_ps[64 * hh : 64 * hh + 64, :],
                            lhsT=ke[:, a, :],
                            rhs=va[:, a, :],
                            start=(a == 0),
                            stop=(a == NT - 1),
                            tile_position=(0, 64 * hh),
                        )
                    kva = sm_p.tile([128, D + 1], BF, tag=f"kva{hh}")
                    nc.vector.tensor_scalar_mul(
                        kva[64 * hh : 64 * hh + 64, :],
                        kv_ps[64 * hh : 64 * hh + 64, :],
                        1.0,
                    )
                    kvas.append(kva)

                for g in range(GROUPS):
                    # -- block-diag scores^T: 8 blocks of [64,64] per head,
                    #    one psum bank per head, 2-slot stationary rotation --
                    sc_list, at_list = [], []
                    for hh in range(2):
                        hp = 64 * hh
                        sc_ps = sc_ps_p.tile(
                            [128, GNT, 64], F32, tag=f"sc{hh}", name="sc_ps",
                            padded_shape=[128, GNT, 128],
                        )
                        for j in range(2 * GNT):
                            a = GNT * g + (j >> 1)
                            half = j & 1
                            b = 2 * a + half
                            nc.tensor.matmul(
                                sc_ps[64 * half : 64 * half + 64, j >> 1, :],
                                lhsT=kt2[hp : hp + 64, 64 * b : 64 * b + 64],
                                rhs=qt2[hp : hp + 64, 64 * b : 64 * b + 64],
                                start=True,
                                stop=True,
                                tile_position=(hp, 64 * half),
                            )
                        at_sb = at_p.tile([128, GNT, 64], BF, tag=f"at{hh}", name="at_sb")
                        nc.scalar.activation(at_sb[:], sc_ps[:], Exp)
                        sc_list.append(sc_ps)
                        at_list.append(at_sb)

                    # -- block-diag out_aug, alternating diagonal slots --
                    da_list = []
                    for hh in range(2):
                        da_ps = da_ps_p.tile(
                            [128, GNT, D + 1], F32, tag=f"da{hh}", name="da_ps",
                            padded_shape=[128, GNT, 128],
                        )
                        for j in range(2 * GNT):
                            i = j >> 1
                            half = j & 1
                            nc.tensor.matmul(
                                da_ps[64 * half : 64 * half + 64, i, :],
                                lhsT=at_list[hh][64 * half : 64 * half + 64, i, :],
                                rhs=vas[hh][64 * half : 64 * half + 64, GNT * g + i, :],
                                start=True,
                                stop=True,
                                tile_position=(64 * half, 64 * half),
                            )
                        da_list.append(da_ps)

                    # -- linear path out_aug, cross-head row-slot rotation --
                    li_list = []
                    for hh in range(2):
                        li_list.append(
                            li_ps_p.tile(
                                [128, GNT, D + 1], F32, tag=f"li{hh}", name="li_ps",
                                padded_shape=[128, GNT, 128],
                            )
                        )
                    for i in range(GNT):
                        a = GNT * g + i
                        for hh in range(2):
                            hp = 64 * hh
                            nc.tensor.matmul(
                                li_list[hh][:, i, :],
                                lhsT=qte2[hp : hp + 64, 128 * a : 128 * a + 128],
                                rhs=kvas[hh][hp : hp + 64, :],
                                start=True,
                                stop=True,
                                tile_position=(hp, 0),
                            )

                    # -- divides + combine --
                    for hh in range(2):
                        li_ps, da_ps, out_h = li_list[hh], da_list[hh], outs[hh]
                        rl = r_p.tile([128, GNT], F32, tag=f"rl{hh}", name="rl")
                        nc.vector.reciprocal(rl[:], li_ps[:, :, D])
                        rd = r_p.tile([128, GNT], F32, tag=f"rd{hh}", name="rd")
                        nc.vector.reciprocal(rd[:], da_ps[:, :, D])
                        t1 = t_p.tile([128, GNT, D], BF, tag=f"t1{hh}", name="t1")
                        nc.vector.tensor_tensor(
                            t1[:], li_ps[:, :, 0:D],
                            rl[:].to_broadcast((128, GNT, D)), op=MUL,
                        )
                        t2 = t_p.tile([128, GNT, D], BF, tag=f"t2{hh}", name="t2")
                        nc.vector.tensor_tensor(
                            t2[:], da_ps[:, :, 0:D],
                            rd[:].to_broadcast((128, GNT, D)), op=MUL,
                        )
                        nc.gpsimd.tensor_tensor(
                            out_h[:, GNT * g : GNT * (g + 1), :], t1[:], t2[:], op=ADD
                        )

                for hh in range(2):
                    h = 2 * p + hh
                    nc.sync.dma_start(out_d[h], outs[hh][:])
    nc.finalize()
    return nc


def _get_nc():
    if "nc" not in _cache:
        _cache["nc"] = _build()
    return _cache["nc"]


def _prep(q, k, v):
    q = np.asarray(q, dtype=np.float32)
    k = np.asarray(k, dtype=np.float32)
    v = np.asarray(v, dtype=np.float32)
    sq = float(np.std(q.astype(np.float64), ddof=1))
    sk = float(np.std(k.astype(np.float64), ddof=1))
    st = math.sqrt((sq * sq * sk * sk - B_CONST) / (2.0 * A_CONST))
    alpha = st / sq
    beta = st / sk

    qf = q.reshape(B * H, N, D)
    kf = k.reshape(B * H, N, D)
    vf = v.reshape(B * H, N, D)
    qt = np.ascontiguousarray(
        (alpha * qf).astype(_BF16).transpose(0, 2, 1).reshape(B * H // 2, 128, N)
    )
    kt = np.ascontiguousarray(
        (kf * (1.0 / (8.0 * alpha)))
        .astype(_BF16)
        .transpose(0, 2, 1)
        .reshape(B * H // 2, 128, N)
    )
    kb = np.ascontiguousarray(
        (beta * kf).astype(_BF16).reshape(B * H, NT, 128, D).transpose(0, 2, 1, 3)
    )
    vb = np.empty((B * H, NT, 128, D + 1), dtype=_BF16)
    vb[..., 0:D] = vf.reshape(B * H, NT, 128, D)
    vb[..., D] = np.float32(2.0)
    vb = np.ascontiguousarray(vb.transpose(0, 2, 1, 3))
    in_maps = []
    for c in range(N_CORES):
        hs = slice(c * HPC, (c + 1) * HPC)
        ps = slice(c * (HPC // 2), (c + 1) * (HPC // 2))
        in_maps.append(
            {
                "qt": np.ascontiguousarray(qt[ps]),
                "kt": np.ascontiguousarray(kt[ps]),
                "kb": np.ascontiguousarray(kb[hs]),
                "vb": np.ascontiguousarray(vb[hs]),
            }
        )
    return in_maps


def run_on_device(in_maps, **kw):
    from concourse.bass_utils import run_bass_kernel_spmd

    return run_bass_kernel_spmd(_get_nc(), in_maps, core_ids=list(range(N_CORES)), **kw)


def kernel(q, k, v):
    in_maps = _prep(q, k, v)
    res = run_on_device(in_maps)
    out = np.concatenate(
        [r["out"].transpose(0, 2, 1, 3).reshape(HPC, N, D) for r in res.results],
        axis=0,
    )
    return np.ascontiguousarray(out.reshape(B, H, N, D)).astype(np.float32)


if __name__ == "__main__":
    nc = _get_nc()
    print("built ok")

